# revision 15
# baseline (speedup 1.0000x reference)
"""Additive attention (Bahdanau) Trainium2 kernel, SPMD over 8 NeuronCores.

Math per batch b (see reference):
    q = queries[b] @ Wq                  [Q=128, H=256]
    k = keys[b]    @ Wk                  [K=1024, H=256]
    scores[i,j] = sum_h wv[h] * tanh(q[i,h] + k[j,h])
    attn = masked_softmax(scores, valid_len[b])
    out[b] = attn @ values[b]            [Q, V=512]

Sharding: sequence-parallel q-striping. Each core takes 16 q-rows of EVERY
batch and only the valid k-range of each batch. Per-core work is perfectly
balanced for any valid_lens; no collectives (softmax is per-q-row).

The kernel is ACT-bound: tanh over ~16*sum(vl)*256 elements per core runs at
1 elem/cycle/lane (153.6 G/s); everything else is scheduled around keeping
the scalar engine 100% busy:
  - group order: smallest batch first (fast ramp while kT streams in),
    middle ascending, 2nd-smallest last (short epilogue tail)
  - one packed [qT|Wq|Wk] input DMA; group-aligned kT segments
  - S-build adds (DVE tensor_scalar, 2x bf16) with a slice of rows on
    GPSIMD to keep DVE under the ACT time; h-tiles merged in one tile so
    each (g,jg) chunk is a single big tanh instruction
  - scores via M=1 matmuls at PE column tiles {0,32,64,96}, two row-halves
    per column tile -> one [128,2,kw] PSUM slab per chunk, one DVE drain
  - exp with accum_out produces the softmax denominator for free (masking
    comes from exp/accum covering exactly [:vl] and host-zeroed values)
  - per-group AV accumulation (start/stop per group) + per-group out DMA:
    no NEG filler needed, tail is only the last (2nd-smallest) group
"""

import os
import sys

import numpy as np

for _p in ("/opt/trn_rl_repo", "/root/.axon_site/_ro/trn_rl_repo"):
    if os.path.isdir(_p) and _p not in sys.path:
        sys.path.insert(0, _p)

os.environ.setdefault("MYCRO_LOCAL_CACHE", "1")

import ml_dtypes  # noqa: E402
from contextlib import ExitStack  # noqa: E402

import concourse.bass as bass  # noqa: E402
import concourse.tile as tile  # noqa: E402
from concourse import bacc, mybir  # noqa: E402
from concourse.bass_utils import run_bass_kernel_spmd  # noqa: E402
from concourse.masks import make_identity  # noqa: E402

BF16 = mybir.dt.bfloat16
F32 = mybir.dt.float32
NP_BF16 = ml_dtypes.bfloat16

B, Q, K, D, H, V = 8, 128, 1024, 512, 256, 512
DC = D // 128   # 4 contraction tiles for the projections
HT = H // 128   # 2 h-tiles
QPC = Q // B    # 16 q-rows per (batch, core)
GP_ROWS = 2     # rows per 8-row chunk whose S-build adds run on GPSIMD


def _kw_template(valid_lens):
    """Group order: smallest batch first (fast pipeline start while kT is
    still streaming), the rest ascending, 2nd-smallest last (short epilogue
    tail). Returns per-GROUP (batch, vl, vl8, kw128, koff)."""
    vls = [max(1, min(K, int(v))) for v in valid_lens]
    by = sorted(range(len(vls)), key=lambda b: (vls[b], b))
    order = [by[0]] + by[2:] + [by[1]]
    vl = [vls[b] for b in order]
    vl8 = [-(-v // 8) * 8 for v in vl]
    kw = [-(-v // 128) * 128 for v in vl]
    koff = np.concatenate([[0], np.cumsum(kw)]).astype(int)
    return order, vl, vl8, kw, koff, int(koff[-1])


def _build_graph(valid_lens):
    _order, vls, vl8s, kws, koff, KEXT = _kw_template(valid_lens)
    nc = bacc.Bacc(
        "TRN2",
        target_bir_lowering=False,
        debug=False,
        num_devices=8,
    )

    # w = [qT | Wq | Wk] packed along columns (all have leading dim D)
    WCOLS = Q + 2 * H
    w = nc.dram_tensor("w", [D, WCOLS], BF16, kind="ExternalInput")
    kT = nc.dram_tensor("kT", [D, KEXT], BF16, kind="ExternalInput")
    vext = nc.dram_tensor("vext", [KEXT, V], BF16, kind="ExternalInput")
    wv2 = nc.dram_tensor("wv2", [128, HT], BF16, kind="ExternalInput")
    out = nc.dram_tensor("out", [Q, V], F32, kind="ExternalOutput")

    # kT segments, group-aligned so each group's projection lands as soon
    # as its own columns arrive (no waiting on later groups' data)
    segs = []
    for g in range(B):
        off = int(koff[g])
        while off < int(koff[g + 1]):
            cw = min(512, int(koff[g + 1]) - off)
            segs.append((off, cw))
            off += cw

    with tile.TileContext(nc) as tc, ExitStack() as ctx:
        singles = ctx.enter_context(tc.tile_pool(name="singles", bufs=1))
        work = ctx.enter_context(tc.tile_pool(name="work", bufs=2))
        psum = ctx.enter_context(tc.tile_pool(name="psum", bufs=1, space="PSUM"))

        # tiny warmup activation so the ~2.7us ACT table load overlaps DMAs
        warm = singles.tile([1, 2], F32)
        nc.vector.memset(warm[:], 0.0)
        nc.scalar.activation(warm[:], warm[:], mybir.ActivationFunctionType.Tanh)

        # ---- input DMAs ------------------------------------------------
        w_sb = singles.tile([128, DC, WCOLS], BF16)
        nc.sync.dma_start(w_sb[:], w.ap().rearrange("(c p) n -> p c n", p=128))
        wv_sb = singles.tile([128, HT], BF16)
        nc.sync.dma_start(wv_sb[:], wv2.ap())
        ident = singles.tile([128, 128], BF16)
        make_identity(nc, ident[:])
        # warm the PE HAM clock gate during the input DMAs so the projection
        # matmuls run at full clock
        pwarm = psum.tile([128, 512], F32, tag="misc", bufs=2, name="pwarm")
        for i in range(8):
            nc.tensor.matmul(
                pwarm[:, :128],
                lhsT=ident[:],
                rhs=ident[:],
                start=(i == 0),
                stop=(i == 7),
            )

        # ---- projections: qh [128, HT, Q] f32, kh [128, HT, KEXT] bf16 --
        qh_sb = singles.tile([128, HT, Q], F32)
        kh_sb = singles.tile([128, HT, KEXT], BF16)

        def _proj(dst, rhs, n, wcol0, ht):
            ps = psum.tile([128, 512], F32, tag="misc", bufs=2, name="ps")
            for dc in range(DC):
                nc.tensor.matmul(
                    ps[:, :n],
                    lhsT=w_sb[:, dc, wcol0 + ht * 128 : wcol0 + (ht + 1) * 128],
                    rhs=rhs(dc),
                    start=(dc == 0),
                    stop=(dc == DC - 1),
                )
            nc.vector.tensor_copy(dst, ps[:, :n])

        for ht in range(HT):
            _proj(qh_sb[:, ht, :], lambda dc: w_sb[:, dc, 0:Q], Q, Q, ht)

        kt_r = kT.ap().rearrange("(c p) k -> p c k", p=128)
        for soff, scw in segs:
            ktc = work.tile([128, DC, 512], BF16, tag="ktc", bufs=3, name="ktc")
            nc.sync.dma_start(ktc[:, :, :scw], kt_r[:, :, soff : soff + scw])
            for ht in range(HT):
                _proj(
                    kh_sb[:, ht, soff : soff + scw],
                    lambda dc: ktc[:, dc, :scw],
                    scw,
                    Q + H,
                    ht,
                )

        # ---- per-group compute ------------------------------------------
        slab = psum.tile([128, 2, 1024], F32, tag="slab", name="slab")
        nc.vector.memset(slab[:], 0.0)
        av = psum.tile([128, V], F32, tag="av", name="av")

        for g in range(B):
            vl, vl8, off = vls[g], vl8s[g], int(koff[g])
            kw128 = kws[g]
            r0g = g * QPC
            # the group's 16 slot rows land at partitions 0:16 (the drain
            # DMA regathers); the whole epilogue runs at base partition 0
            # and the out DMA scatters rows back to their slot offset
            scores_g = work.tile([128, kw128], F32, tag="scores", name="scores_g")
            if vl < kw128:
                # pad columns feed the denominator accum -> push them to ~0
                nc.vector.memset(scores_g[0:QPC, vl:kw128], -60.0)
            for jg in range(QPC // 8):
                r0 = g * QPC + jg * 8
                st = work.tile(
                    [128, HT, 8, vl8], BF16, tag="st", bufs=3, name="st"
                )
                for j in range(8):
                    eng = nc.gpsimd if j >= 8 - GP_ROWS else nc.vector
                    for ht in range(HT):
                        eng.tensor_scalar_add(
                            st[:, ht, j, :],
                            kh_sb[:, ht, off : off + vl8],
                            qh_sb[:, ht, r0 + j : r0 + j + 1],
                        )
                if g == 0 and jg == 0:
                    # split so ACT starts after only 4 rows' worth of adds
                    nc.scalar.activation(
                        st[:, :, 0:4, :], st[:, :, 0:4, :],
                        mybir.ActivationFunctionType.Tanh,
                    )
                    nc.scalar.activation(
                        st[:, :, 4:8, :], st[:, :, 4:8, :],
                        mybir.ActivationFunctionType.Tanh,
                    )
                else:
                    nc.scalar.activation(
                        st[:], st[:], mybir.ActivationFunctionType.Tanh
                    )
                # scores: M=1 matmuls; row j -> PE column tile 32*(j//2),
                # slab half j%2 (so the strided drain lands rows in order)
                for ht in range(HT):
                    for j in range(8):
                        j4, half = j >> 1, j & 1
                        for c in range(0, vl8, 512):
                            cw = min(512, vl8 - c)
                            nc.tensor.matmul(
                                slab[32 * j4 : 32 * j4 + 1, half, c : c + cw],
                                lhsT=wv_sb[:, ht : ht + 1],
                                rhs=st[:, ht, j, c : c + cw],
                                start=(ht == 0),
                                stop=(ht == 1),
                                tile_position=(0, 32 * j4),
                            )
                stage = work.tile(
                    [128, 2, vl8], F32, tag="stage", bufs=2, name="stage"
                )
                nc.vector.tensor_copy(stage[:, :, :vl], slab[:, :, :vl])
                nc.sync.dma_start(
                    scores_g[r0 - r0g : r0 - r0g + 8, :vl],
                    stage[0:128:32, :, :vl],
                )

            # epilogue: exp (+denominator via accum) on the group's 16 slot
            # rows, 16-wide transposes, AV with per-group PSUM accumulation
            # into av[0:16], scaled rows DMA'd out to their slot offset
            l_g = work.tile([128, 1], F32, tag="l", name="l_g")
            exp_g = work.tile([128, kw128], BF16, tag="exp", name="exp_g")
            nc.scalar.activation(
                exp_g[0:QPC, :],
                scores_g[0:QPC, :],
                mybir.ActivationFunctionType.Exp,
                accum_out=l_g[0:QPC],
            )
            rl = work.tile([128, 1], F32, tag="rl", name="rl")
            nc.vector.reciprocal(rl[0:QPC], l_g[0:QPC])
            ntile = kw128 // 128
            for t in range(ntile):
                kt = off // 128 + t
                vt = work.tile([128, V], BF16, tag="vt", bufs=4, name="vt")
                nc.sync.dma_start(vt[:], vext[kt * 128 : (kt + 1) * 128, :])
                pt = psum.tile([128, 128], BF16, tag="misc", bufs=2, name="pt")
                nc.tensor.transpose(
                    pt[:, 0:QPC],
                    exp_g[0:QPC, t * 128 : (t + 1) * 128],
                    ident[0:QPC, 0:QPC],
                )
                expt = work.tile([128, QPC], BF16, tag="expT", bufs=4, name="expt")
                nc.vector.tensor_copy(expt[:], pt[:, 0:QPC])
                nc.tensor.matmul(
                    av[0:QPC, 0:V],
                    lhsT=expt[:],
                    rhs=vt[:],
                    start=(t == 0),
                    stop=(t == ntile - 1),
                )
            osb = work.tile([128, V], F32, tag="osb", bufs=2, name="osb")
            nc.vector.tensor_scalar_mul(
                osb[0:QPC, :], av[0:QPC, 0:V], rl[0:QPC]
            )
            nc.sync.dma_start(out.ap()[r0g : r0g + QPC, :], osb[0:QPC, :])

    nc.compile()
    return nc


_CACHE = {}


def _install_profile_shim():
    """Provide antenv.axon_hooks (absent in this image) so
    run_bass_kernel_spmd(trace=True) can capture NTFF profiles through
    libaxon_pjrt.so, mirroring trn_agent_boot's bootstrap."""
    import types

    if "antenv.axon_hooks" not in sys.modules:
        mod = types.ModuleType("antenv.axon_hooks")
        state = {}
        mod.set_axon_ntff_profile_hook = lambda h: state.__setitem__("h", h)
        mod.get_axon_ntff_profile_hook = lambda: state.get("h")
        sys.modules["antenv.axon_hooks"] = mod
        import antenv

        antenv.axon_hooks = mod
        if "/root/.axon_site" not in sys.path:
            sys.path.insert(0, "/root/.axon_site")
        from trn_agent_boot.trn_boot import _ntff_profile_via_ctypes

        hook = _ntff_profile_via_ctypes("/opt/axon/libaxon_pjrt.so")
        mod.set_axon_ntff_profile_hook(hook)

        import concourse.bass_utils as bu

        orig_upload = bu.upload_artifacts

        def _safe_upload(tmpdir):
            try:
                return orig_upload(tmpdir)
            except Exception:
                return f"local:{tmpdir}"

        bu.upload_artifacts = _safe_upload


def _get_graph(valid_lens):
    key = tuple(int(v) for v in valid_lens)
    if _CACHE.get("key") != key:
        _CACHE["nc"] = _build_graph(valid_lens)
        _CACHE["key"] = key
    return _CACHE["nc"]


def _make_in_maps(queries, keys, values, valid_lens):
    order, vls, _vl8s, kws, koff, KEXT = _kw_template(valid_lens)
    kT = np.zeros((D, KEXT), dtype=np.float32)
    vext = np.zeros((KEXT, V), dtype=np.float32)
    for g, b in enumerate(order):
        vl, kw, off = vls[g], kws[g], int(koff[g])
        kT[:, off : off + kw] = keys[b, :kw].T
        vext[off : off + vl, :] = values[b, :vl]
    kT_bf = kT.astype(NP_BF16)
    vext_bf = vext.astype(NP_BF16)
    in_maps = []
    for c in range(B):
        qrows = np.concatenate(
            [queries[b, c * QPC : (c + 1) * QPC] for b in order], axis=0
        )  # [128, D]; slot 16*g + r = (batch order[g], row 16*c + r)
        w = np.concatenate(
            [qrows.T, _CACHE["wq_f"], _CACHE["wk_f"]], axis=1
        )  # [D, Q+2H]
        in_maps.append(
            {
                "w": w.astype(NP_BF16),
                "kT": kT_bf,
                "vext": vext_bf,
                "wv2": _CACHE["wv2_bf"],
            }
        )
    return in_maps


def kernel(
    queries, keys, values, valid_lens, Wq, Wk, wv, _profile=False, **_unused
):
    queries = np.asarray(queries, dtype=np.float32)
    keys = np.asarray(keys, dtype=np.float32)
    values = np.asarray(values, dtype=np.float32)
    valid_lens = np.asarray(valid_lens)
    _CACHE["wq_f"] = np.asarray(Wq, np.float32)
    _CACHE["wk_f"] = np.asarray(Wk, np.float32)
    _CACHE["wv2_bf"] = (
        np.asarray(wv, np.float32).reshape(HT, 128).T.copy().astype(NP_BF16)
    )

    nc = _get_graph(valid_lens)
    in_maps = _make_in_maps(queries, keys, values, valid_lens)
    kwargs = {}
    if _profile:
        _install_profile_shim()
        tdir = "/root/problem/trace_out"
        os.makedirs(tdir, exist_ok=True)
        kwargs["tmpdir"] = tdir
    res = run_bass_kernel_spmd(
        nc, in_maps, core_ids=list(range(B)), trace=_profile, **kwargs
    )
    order = _kw_template(valid_lens)[0]
    out = np.zeros((B, Q, V), dtype=np.float32)
    for c in range(B):
        oc = np.asarray(res.results[c]["out"], dtype=np.float32)
        for g, b in enumerate(order):
            out[b, c * QPC : (c + 1) * QPC] = oc[g * QPC : (g + 1) * QPC]
    if _profile:
        _CACHE["last_result"] = res
    return out


# revision 17
# speedup vs baseline: 4.1926x; 4.1926x over previous
"""Additive attention (Bahdanau) Trainium2 kernel, SPMD over 8 NeuronCores.

Math per batch b (see reference):
    q = queries[b] @ Wq                  [Q=128, H=256]
    k = keys[b]    @ Wk                  [K=1024, H=256]
    scores[i,j] = sum_h wv[h] * tanh(q[i,h] + k[j,h])
    attn = masked_softmax(scores, valid_len[b])
    out[b] = attn @ values[b]            [Q, V=512]

Sharding: sequence-parallel q-striping. Each core takes 16 q-rows of EVERY
batch and only the valid k-range of each batch. Per-core work is perfectly
balanced for any valid_lens; no collectives (softmax is per-q-row).

The kernel is ACT-bound: tanh over ~16*sum(vl)*256 elements per core runs at
1 elem/cycle/lane (153.6 G/s); everything else is scheduled around keeping
the scalar engine 100% busy:
  - group order: smallest batch first (fast ramp while kT streams in),
    middle ascending, 2nd-smallest last (short epilogue tail)
  - one packed [qT|Wq|Wk] input DMA; group-aligned kT segments
  - S-build adds (DVE tensor_scalar, 2x bf16) with a slice of rows on
    GPSIMD to keep DVE under the ACT time; h-tiles merged in one tile so
    each (g,jg) chunk is a single big tanh instruction
  - scores via M=1 matmuls at PE column tiles {0,32,64,96}, two row-halves
    per column tile -> one [128,2,kw] PSUM slab per chunk, one DVE drain
  - exp with accum_out produces the softmax denominator for free (masking
    comes from exp/accum covering exactly [:vl] and host-zeroed values)
  - per-group AV accumulation (start/stop per group) + per-group out DMA:
    no NEG filler needed, tail is only the last (2nd-smallest) group
"""

import os
import sys

import numpy as np

for _p in ("/opt/trn_rl_repo", "/root/.axon_site/_ro/trn_rl_repo"):
    if os.path.isdir(_p) and _p not in sys.path:
        sys.path.insert(0, _p)

os.environ.setdefault("MYCRO_LOCAL_CACHE", "1")

import ml_dtypes  # noqa: E402
from contextlib import ExitStack  # noqa: E402

import concourse.bass as bass  # noqa: E402
import concourse.tile as tile  # noqa: E402
from concourse import bacc, mybir  # noqa: E402
from concourse.bass_utils import run_bass_kernel_spmd  # noqa: E402
from concourse.masks import make_identity  # noqa: E402

BF16 = mybir.dt.bfloat16
F32 = mybir.dt.float32
NP_BF16 = ml_dtypes.bfloat16

B, Q, K, D, H, V = 8, 128, 1024, 512, 256, 512
DC = D // 128   # 4 contraction tiles for the projections
HT = H // 128   # 2 h-tiles
QPC = Q // B    # 16 q-rows per (batch, core)
ACT_ROWS = 1    # rows per 8-row chunk done as tanh(kh + bias) on ACT
                # (costs +2x224cyc on ACT, saves 2x(58+vl/2)cyc on DVE --
                # balances the two near-critical engines)


def _kw_template(valid_lens):
    """Group order: smallest batch first (fast pipeline start while kT is
    still streaming), the rest ascending, 2nd-smallest last (short epilogue
    tail). Returns per-GROUP (batch, vl, vl8, kw128, koff)."""
    vls = [max(1, min(K, int(v))) for v in valid_lens]
    by = sorted(range(len(vls)), key=lambda b: (vls[b], b))
    order = [by[0]] + by[2:] + [by[1]]
    vl = [vls[b] for b in order]
    vl8 = [-(-v // 8) * 8 for v in vl]
    kw = [-(-v // 128) * 128 for v in vl]
    koff = np.concatenate([[0], np.cumsum(kw)]).astype(int)
    return order, vl, vl8, kw, koff, int(koff[-1])


def _build_graph(valid_lens):
    _order, vls, vl8s, kws, koff, KEXT = _kw_template(valid_lens)
    nc = bacc.Bacc(
        "TRN2",
        target_bir_lowering=False,
        debug=False,
        num_devices=8,
    )

    # w = [qT | Wq | Wk] packed along columns (all have leading dim D)
    WCOLS = Q + 2 * H
    w = nc.dram_tensor("w", [D, WCOLS], BF16, kind="ExternalInput")
    kT = nc.dram_tensor("kT", [D, KEXT], BF16, kind="ExternalInput")
    vext = nc.dram_tensor("vext", [KEXT, V], BF16, kind="ExternalInput")
    wv2 = nc.dram_tensor("wv2", [128, HT], BF16, kind="ExternalInput")
    out = nc.dram_tensor("out", [Q, V], F32, kind="ExternalOutput")

    # kT segments, group-aligned so each group's projection lands as soon
    # as its own columns arrive (no waiting on later groups' data)
    segs = []
    for g in range(B):
        off = int(koff[g])
        while off < int(koff[g + 1]):
            cw = min(512, int(koff[g + 1]) - off)
            segs.append((off, cw))
            off += cw

    with tile.TileContext(nc) as tc, ExitStack() as ctx:
        singles = ctx.enter_context(tc.tile_pool(name="singles", bufs=1))
        work = ctx.enter_context(tc.tile_pool(name="work", bufs=2))
        psum = ctx.enter_context(tc.tile_pool(name="psum", bufs=1, space="PSUM"))

        # tiny warmup activation so the ~2.7us ACT table load overlaps DMAs
        warm = singles.tile([1, 2], F32)
        nc.vector.memset(warm[:], 0.0)
        nc.scalar.activation(warm[:], warm[:], mybir.ActivationFunctionType.Tanh)

        # ---- input DMAs ------------------------------------------------
        w_sb = singles.tile([128, DC, WCOLS], BF16)
        nc.sync.dma_start(w_sb[:], w.ap().rearrange("(c p) n -> p c n", p=128))
        wv_sb = singles.tile([128, HT], BF16)
        nc.sync.dma_start(wv_sb[:], wv2.ap())
        ident = singles.tile([128, 128], BF16)
        make_identity(nc, ident[:])
        # warm the PE HAM clock gate during the input DMAs so the projection
        # matmuls run at full clock
        pwarm = psum.tile([128, 512], F32, tag="misc", bufs=2, name="pwarm")
        for i in range(8):
            nc.tensor.matmul(
                pwarm[:, :128],
                lhsT=ident[:],
                rhs=ident[:],
                start=(i == 0),
                stop=(i == 7),
            )

        # ---- projections: qh [128, HT, Q] f32, kh [128, HT, KEXT] bf16 --
        qh_sb = singles.tile([128, HT, Q], F32)
        kh_sb = singles.tile([128, HT, KEXT], BF16)

        def _proj(dst, rhs, n, wcol0, ht):
            ps = psum.tile([128, 512], F32, tag="misc", bufs=2, name="ps")
            for dc in range(DC):
                nc.tensor.matmul(
                    ps[:, :n],
                    lhsT=w_sb[:, dc, wcol0 + ht * 128 : wcol0 + (ht + 1) * 128],
                    rhs=rhs(dc),
                    start=(dc == 0),
                    stop=(dc == DC - 1),
                )
            nc.vector.tensor_copy(dst, ps[:, :n])

        for ht in range(HT):
            _proj(qh_sb[:, ht, :], lambda dc: w_sb[:, dc, 0:Q], Q, Q, ht)

        kt_r = kT.ap().rearrange("(c p) k -> p c k", p=128)
        for soff, scw in segs:
            ktc = work.tile([128, DC, 512], BF16, tag="ktc", bufs=3, name="ktc")
            nc.sync.dma_start(ktc[:, :, :scw], kt_r[:, :, soff : soff + scw])
            for ht in range(HT):
                _proj(
                    kh_sb[:, ht, soff : soff + scw],
                    lambda dc: ktc[:, dc, :scw],
                    scw,
                    Q + H,
                    ht,
                )

        # ---- per-group compute ------------------------------------------
        slab = psum.tile([128, 2, 1024], F32, tag="slab", name="slab")
        nc.vector.memset(slab[:], 0.0)
        av = psum.tile([128, V], F32, tag="av", name="av")

        for g in range(B):
            vl, vl8, off = vls[g], vl8s[g], int(koff[g])
            kw128 = kws[g]
            r0g = g * QPC
            # the group's 16 slot rows land at partitions 0:16 (the drain
            # DMA regathers); the whole epilogue runs at base partition 0
            # and the out DMA scatters rows back to their slot offset
            scores_g = work.tile([128, kw128], F32, tag="scores", name="scores_g")
            if vl < kw128:
                # pad columns feed the denominator accum -> push them to ~0
                nc.vector.memset(scores_g[0:QPC, vl:kw128], -60.0)
            for jg in range(QPC // 8):
                r0 = g * QPC + jg * 8
                st = work.tile(
                    [128, HT, 8, vl8], BF16, tag="st", bufs=3, name="st"
                )
                nd = 8 - ACT_ROWS  # rows 0..nd added on DVE, rest via bias
                for j in range(nd):
                    for ht in range(HT):
                        nc.vector.tensor_scalar_add(
                            st[:, ht, j, :],
                            kh_sb[:, ht, off : off + vl8],
                            qh_sb[:, ht, r0 + j : r0 + j + 1],
                        )
                # bias rows first: their input (kh) is ready before the DVE
                # adds finish, so ACT can run them while DVE works
                for j in range(nd, 8):
                    for ht in range(HT):
                        nc.scalar.activation(
                            st[:, ht, j, :],
                            kh_sb[:, ht, off : off + vl8],
                            mybir.ActivationFunctionType.Tanh,
                            bias=qh_sb[:, ht, r0 + j : r0 + j + 1],
                        )
                if g == 0 and jg == 0:
                    # split so ACT starts after only 4 rows' worth of adds
                    nc.scalar.activation(
                        st[:, :, 0:4, :], st[:, :, 0:4, :],
                        mybir.ActivationFunctionType.Tanh,
                    )
                    nc.scalar.activation(
                        st[:, :, 4:nd, :], st[:, :, 4:nd, :],
                        mybir.ActivationFunctionType.Tanh,
                    )
                else:
                    nc.scalar.activation(
                        st[:, :, 0:nd, :], st[:, :, 0:nd, :],
                        mybir.ActivationFunctionType.Tanh,
                    )
                # scores: M=1 matmuls; row j -> PE column tile 32*(j//2),
                # slab half j%2 (so the strided drain lands rows in order)
                for ht in range(HT):
                    for j in range(8):
                        j4, half = j >> 1, j & 1
                        for c in range(0, vl8, 512):
                            cw = min(512, vl8 - c)
                            nc.tensor.matmul(
                                slab[32 * j4 : 32 * j4 + 1, half, c : c + cw],
                                lhsT=wv_sb[:, ht : ht + 1],
                                rhs=st[:, ht, j, c : c + cw],
                                start=(ht == 0),
                                stop=(ht == 1),
                                tile_position=(0, 32 * j4),
                            )
                stage = work.tile(
                    [128, 2, vl8], F32, tag="stage", bufs=2, name="stage"
                )
                nc.vector.tensor_copy(stage[:, :, :vl], slab[:, :, :vl])
                nc.sync.dma_start(
                    scores_g[r0 - r0g : r0 - r0g + 8, :vl],
                    stage[0:128:32, :, :vl],
                )

            # epilogue: exp (+denominator via accum) on the group's 16 slot
            # rows, 16-wide transposes, AV with per-group PSUM accumulation
            # into av[0:16], scaled rows DMA'd out to their slot offset
            l_g = work.tile([128, 1], F32, tag="l", name="l_g")
            exp_g = work.tile([128, kw128], BF16, tag="exp", name="exp_g")
            nc.scalar.activation(
                exp_g[0:QPC, :],
                scores_g[0:QPC, :],
                mybir.ActivationFunctionType.Exp,
                accum_out=l_g[0:QPC],
            )
            rl = work.tile([128, 1], F32, tag="rl", name="rl")
            nc.vector.reciprocal(rl[0:QPC], l_g[0:QPC])
            ntile = kw128 // 128
            for t in range(ntile):
                kt = off // 128 + t
                vt = work.tile([128, V], BF16, tag="vt", bufs=4, name="vt")
                nc.sync.dma_start(vt[:], vext[kt * 128 : (kt + 1) * 128, :])
                pt = psum.tile([128, 128], BF16, tag="misc", bufs=2, name="pt")
                nc.tensor.transpose(
                    pt[:, 0:QPC],
                    exp_g[0:QPC, t * 128 : (t + 1) * 128],
                    ident[0:QPC, 0:QPC],
                )
                expt = work.tile([128, QPC], BF16, tag="expT", bufs=4, name="expt")
                nc.vector.tensor_copy(expt[:], pt[:, 0:QPC])
                nc.tensor.matmul(
                    av[0:QPC, 0:V],
                    lhsT=expt[:],
                    rhs=vt[:],
                    start=(t == 0),
                    stop=(t == ntile - 1),
                )
            osb = work.tile([128, V], F32, tag="osb", bufs=2, name="osb")
            nc.vector.tensor_scalar_mul(
                osb[0:QPC, :], av[0:QPC, 0:V], rl[0:QPC]
            )
            nc.sync.dma_start(out.ap()[r0g : r0g + QPC, :], osb[0:QPC, :])

    nc.compile()
    return nc


_CACHE = {}


def _install_profile_shim():
    """Provide antenv.axon_hooks (absent in this image) so
    run_bass_kernel_spmd(trace=True) can capture NTFF profiles through
    libaxon_pjrt.so, mirroring trn_agent_boot's bootstrap."""
    import types

    if "antenv.axon_hooks" not in sys.modules:
        mod = types.ModuleType("antenv.axon_hooks")
        state = {}
        mod.set_axon_ntff_profile_hook = lambda h: state.__setitem__("h", h)
        mod.get_axon_ntff_profile_hook = lambda: state.get("h")
        sys.modules["antenv.axon_hooks"] = mod
        import antenv

        antenv.axon_hooks = mod
        if "/root/.axon_site" not in sys.path:
            sys.path.insert(0, "/root/.axon_site")
        from trn_agent_boot.trn_boot import _ntff_profile_via_ctypes

        hook = _ntff_profile_via_ctypes("/opt/axon/libaxon_pjrt.so")
        mod.set_axon_ntff_profile_hook(hook)

        import concourse.bass_utils as bu

        orig_upload = bu.upload_artifacts

        def _safe_upload(tmpdir):
            try:
                return orig_upload(tmpdir)
            except Exception:
                return f"local:{tmpdir}"

        bu.upload_artifacts = _safe_upload


def _get_graph(valid_lens):
    key = tuple(int(v) for v in valid_lens)
    if _CACHE.get("key") != key:
        _CACHE["nc"] = _build_graph(valid_lens)
        _CACHE["key"] = key
    return _CACHE["nc"]


def _make_in_maps(queries, keys, values, valid_lens):
    order, vls, _vl8s, kws, koff, KEXT = _kw_template(valid_lens)
    kT = np.zeros((D, KEXT), dtype=np.float32)
    vext = np.zeros((KEXT, V), dtype=np.float32)
    for g, b in enumerate(order):
        vl, kw, off = vls[g], kws[g], int(koff[g])
        kT[:, off : off + kw] = keys[b, :kw].T
        vext[off : off + vl, :] = values[b, :vl]
    kT_bf = kT.astype(NP_BF16)
    vext_bf = vext.astype(NP_BF16)
    in_maps = []
    for c in range(B):
        qrows = np.concatenate(
            [queries[b, c * QPC : (c + 1) * QPC] for b in order], axis=0
        )  # [128, D]; slot 16*g + r = (batch order[g], row 16*c + r)
        w = np.concatenate(
            [qrows.T, _CACHE["wq_f"], _CACHE["wk_f"]], axis=1
        )  # [D, Q+2H]
        in_maps.append(
            {
                "w": w.astype(NP_BF16),
                "kT": kT_bf,
                "vext": vext_bf,
                "wv2": _CACHE["wv2_bf"],
            }
        )
    return in_maps


def kernel(
    queries, keys, values, valid_lens, Wq, Wk, wv, _profile=False, **_unused
):
    queries = np.asarray(queries, dtype=np.float32)
    keys = np.asarray(keys, dtype=np.float32)
    values = np.asarray(values, dtype=np.float32)
    valid_lens = np.asarray(valid_lens)
    _CACHE["wq_f"] = np.asarray(Wq, np.float32)
    _CACHE["wk_f"] = np.asarray(Wk, np.float32)
    _CACHE["wv2_bf"] = (
        np.asarray(wv, np.float32).reshape(HT, 128).T.copy().astype(NP_BF16)
    )

    nc = _get_graph(valid_lens)
    in_maps = _make_in_maps(queries, keys, values, valid_lens)
    kwargs = {}
    if _profile:
        _install_profile_shim()
        tdir = "/root/problem/trace_out"
        os.makedirs(tdir, exist_ok=True)
        kwargs["tmpdir"] = tdir
    res = run_bass_kernel_spmd(
        nc, in_maps, core_ids=list(range(B)), trace=_profile, **kwargs
    )
    order = _kw_template(valid_lens)[0]
    out = np.zeros((B, Q, V), dtype=np.float32)
    for c in range(B):
        oc = np.asarray(res.results[c]["out"], dtype=np.float32)
        for g, b in enumerate(order):
            out[b, c * QPC : (c + 1) * QPC] = oc[g * QPC : (g + 1) * QPC]
    if _profile:
        _CACHE["last_result"] = res
    return out


# revision 18
# speedup vs baseline: 4.2652x; 1.0173x over previous
"""Additive attention (Bahdanau) Trainium2 kernel, SPMD over 8 NeuronCores.

Math per batch b (see reference):
    q = queries[b] @ Wq                  [Q=128, H=256]
    k = keys[b]    @ Wk                  [K=1024, H=256]
    scores[i,j] = sum_h wv[h] * tanh(q[i,h] + k[j,h])
    attn = masked_softmax(scores, valid_len[b])
    out[b] = attn @ values[b]            [Q, V=512]

Sharding: sequence-parallel q-striping. Each core takes 16 q-rows of EVERY
batch and only the valid k-range of each batch. Per-core work is perfectly
balanced for any valid_lens; no collectives (softmax is per-q-row).

The kernel is ACT-bound: tanh over ~16*sum(vl)*256 elements per core runs at
1 elem/cycle/lane (153.6 G/s); everything else is scheduled around keeping
the scalar engine 100% busy:
  - group order: smallest batch first (fast ramp while kT streams in),
    middle ascending, 2nd-smallest last (short epilogue tail)
  - one packed [qT|Wq|Wk] input DMA; group-aligned kT segments
  - S-build adds (DVE tensor_scalar, 2x bf16) with a slice of rows on
    GPSIMD to keep DVE under the ACT time; h-tiles merged in one tile so
    each (g,jg) chunk is a single big tanh instruction
  - scores via M=1 matmuls at PE column tiles {0,32,64,96}, two row-halves
    per column tile -> one [128,2,kw] PSUM slab per chunk, one DVE drain
  - exp with accum_out produces the softmax denominator for free (masking
    comes from exp/accum covering exactly [:vl] and host-zeroed values)
  - per-group AV accumulation (start/stop per group) + per-group out DMA:
    no NEG filler needed, tail is only the last (2nd-smallest) group
"""

import os
import sys

import numpy as np

for _p in ("/opt/trn_rl_repo", "/root/.axon_site/_ro/trn_rl_repo"):
    if os.path.isdir(_p) and _p not in sys.path:
        sys.path.insert(0, _p)

os.environ.setdefault("MYCRO_LOCAL_CACHE", "1")

import ml_dtypes  # noqa: E402
from contextlib import ExitStack  # noqa: E402

import concourse.bass as bass  # noqa: E402
import concourse.tile as tile  # noqa: E402
from concourse import bacc, mybir  # noqa: E402
from concourse.bass_utils import run_bass_kernel_spmd  # noqa: E402
from concourse.masks import make_identity  # noqa: E402

BF16 = mybir.dt.bfloat16
F32 = mybir.dt.float32
NP_BF16 = ml_dtypes.bfloat16

B, Q, K, D, H, V = 8, 128, 1024, 512, 256, 512
DC = D // 128   # 4 contraction tiles for the projections
HT = H // 128   # 2 h-tiles
QPC = Q // B    # 16 q-rows per (batch, core)
ACT_ROWS = 0    # rows per 8-row chunk done as tanh(kh + bias) on ACT
                # (costs +2x224cyc on ACT, saves 2x(58+vl/2)cyc on DVE --
                # balances the two near-critical engines)


def _kw_template(valid_lens):
    """Group order: smallest batch first (fast pipeline start while kT is
    still streaming), the rest ascending, 2nd-smallest last (short epilogue
    tail). Returns per-GROUP (batch, vl, vl8, kw128, koff)."""
    vls = [max(1, min(K, int(v))) for v in valid_lens]
    by = sorted(range(len(vls)), key=lambda b: (vls[b], b))
    order = [by[0]] + by[2:] + [by[1]]
    vl = [vls[b] for b in order]
    vl8 = [-(-v // 8) * 8 for v in vl]
    kw = [-(-v // 128) * 128 for v in vl]
    koff = np.concatenate([[0], np.cumsum(kw)]).astype(int)
    return order, vl, vl8, kw, koff, int(koff[-1])


def _build_graph(valid_lens):
    _order, vls, vl8s, kws, koff, KEXT = _kw_template(valid_lens)
    nc = bacc.Bacc(
        "TRN2",
        target_bir_lowering=False,
        debug=False,
        num_devices=8,
    )

    # w = [qT | Wq | Wk] packed along columns (all have leading dim D)
    WCOLS = Q + 2 * H
    w = nc.dram_tensor("w", [D, WCOLS], BF16, kind="ExternalInput")
    kT = nc.dram_tensor("kT", [D, KEXT], BF16, kind="ExternalInput")
    vext = nc.dram_tensor("vext", [KEXT, V], BF16, kind="ExternalInput")
    wv2 = nc.dram_tensor("wv2", [128, HT], BF16, kind="ExternalInput")
    out = nc.dram_tensor("out", [Q, V], F32, kind="ExternalOutput")

    # kT segments, group-aligned so each group's projection lands as soon
    # as its own columns arrive (no waiting on later groups' data)
    segs = []
    for g in range(B):
        off = int(koff[g])
        while off < int(koff[g + 1]):
            cw = min(512, int(koff[g + 1]) - off)
            segs.append((off, cw))
            off += cw

    with tile.TileContext(nc) as tc, ExitStack() as ctx:
        singles = ctx.enter_context(tc.tile_pool(name="singles", bufs=1))
        work = ctx.enter_context(tc.tile_pool(name="work", bufs=2))
        psum = ctx.enter_context(tc.tile_pool(name="psum", bufs=1, space="PSUM"))

        # tiny warmup activation so the ~2.7us ACT table load overlaps DMAs
        warm = singles.tile([1, 2], F32)
        nc.vector.memset(warm[:], 0.0)
        nc.scalar.activation(warm[:], warm[:], mybir.ActivationFunctionType.Tanh)

        # ---- input DMAs ------------------------------------------------
        w_sb = singles.tile([128, DC, WCOLS], BF16)
        nc.sync.dma_start(w_sb[:], w.ap().rearrange("(c p) n -> p c n", p=128))
        wv_sb = singles.tile([128, HT], BF16)
        nc.sync.dma_start(wv_sb[:], wv2.ap())
        ident = singles.tile([128, 128], BF16)
        make_identity(nc, ident[:])
        # warm the PE HAM clock gate during the input DMAs so the projection
        # matmuls run at full clock
        pwarm = psum.tile([128, 512], F32, tag="misc", bufs=2, name="pwarm")
        for i in range(8):
            nc.tensor.matmul(
                pwarm[:, :128],
                lhsT=ident[:],
                rhs=ident[:],
                start=(i == 0),
                stop=(i == 7),
            )

        # ---- projections: qh [128, HT, Q] f32, kh [128, HT, KEXT] bf16 --
        qh_sb = singles.tile([128, HT, Q], F32)
        kh_sb = singles.tile([128, HT, KEXT], BF16)

        def _proj(dst, rhs, n, wcol0, ht):
            ps = psum.tile([128, 512], F32, tag="misc", bufs=2, name="ps")
            for dc in range(DC):
                nc.tensor.matmul(
                    ps[:, :n],
                    lhsT=w_sb[:, dc, wcol0 + ht * 128 : wcol0 + (ht + 1) * 128],
                    rhs=rhs(dc),
                    start=(dc == 0),
                    stop=(dc == DC - 1),
                )
            nc.vector.tensor_copy(dst, ps[:, :n])

        for ht in range(HT):
            _proj(qh_sb[:, ht, :], lambda dc: w_sb[:, dc, 0:Q], Q, Q, ht)

        kt_r = kT.ap().rearrange("(c p) k -> p c k", p=128)
        for soff, scw in segs:
            ktc = work.tile([128, DC, 512], BF16, tag="ktc", bufs=2, name="ktc")
            nc.sync.dma_start(ktc[:, :, :scw], kt_r[:, :, soff : soff + scw])
            for ht in range(HT):
                _proj(
                    kh_sb[:, ht, soff : soff + scw],
                    lambda dc: ktc[:, dc, :scw],
                    scw,
                    Q + H,
                    ht,
                )

        # ---- per-group compute ------------------------------------------
        slab = psum.tile([128, 2, 1024], F32, tag="slab", name="slab")
        nc.vector.memset(slab[:], 0.0)
        av = psum.tile([128, V], F32, tag="av", name="av")

        for g in range(B):
            vl, vl8, off = vls[g], vl8s[g], int(koff[g])
            kw128 = kws[g]
            r0g = g * QPC
            # the group's 16 slot rows land at partitions 0:16 (the drain
            # DMA regathers); the whole epilogue runs at base partition 0
            # and the out DMA scatters rows back to their slot offset
            scores_g = work.tile([128, kw128], F32, tag="scores", name="scores_g")
            if vl < kw128:
                # pad columns feed the denominator accum -> push them to ~0
                nc.vector.memset(scores_g[0:QPC, vl:kw128], -60.0)
            for jg in range(QPC // 8):
                r0 = g * QPC + jg * 8
                st = work.tile(
                    [128, HT, 8, vl8], BF16, tag="st", bufs=3, name="st"
                )
                nd = 8 - ACT_ROWS  # rows 0..nd added on DVE, rest via bias
                for j in range(nd):
                    for ht in range(HT):
                        nc.vector.tensor_scalar_add(
                            st[:, ht, j, :],
                            kh_sb[:, ht, off : off + vl8],
                            qh_sb[:, ht, r0 + j : r0 + j + 1],
                        )
                # bias rows first: their input (kh) is ready before the DVE
                # adds finish, so ACT can run them while DVE works
                for j in range(nd, 8):
                    for ht in range(HT):
                        nc.scalar.activation(
                            st[:, ht, j, :],
                            kh_sb[:, ht, off : off + vl8],
                            mybir.ActivationFunctionType.Tanh,
                            bias=qh_sb[:, ht, r0 + j : r0 + j + 1],
                        )
                if g == 0 and jg == 0:
                    # split so ACT starts after only 4 rows' worth of adds
                    nc.scalar.activation(
                        st[:, :, 0:4, :], st[:, :, 0:4, :],
                        mybir.ActivationFunctionType.Tanh,
                    )
                    nc.scalar.activation(
                        st[:, :, 4:nd, :], st[:, :, 4:nd, :],
                        mybir.ActivationFunctionType.Tanh,
                    )
                else:
                    nc.scalar.activation(
                        st[:, :, 0:nd, :], st[:, :, 0:nd, :],
                        mybir.ActivationFunctionType.Tanh,
                    )
                # scores: M=1 matmuls; row j -> PE column tile 32*(j//2),
                # slab half j%2 (so the strided drain lands rows in order)
                for ht in range(HT):
                    for j in range(8):
                        j4, half = j >> 1, j & 1
                        for c in range(0, vl8, 512):
                            cw = min(512, vl8 - c)
                            nc.tensor.matmul(
                                slab[32 * j4 : 32 * j4 + 1, half, c : c + cw],
                                lhsT=wv_sb[:, ht : ht + 1],
                                rhs=st[:, ht, j, c : c + cw],
                                start=(ht == 0),
                                stop=(ht == 1),
                                tile_position=(0, 32 * j4),
                            )
                stage = work.tile(
                    [128, 2, vl8], F32, tag="stage", bufs=2, name="stage"
                )
                nc.vector.tensor_copy(stage[:, :, :vl], slab[:, :, :vl])
                nc.sync.dma_start(
                    scores_g[r0 - r0g : r0 - r0g + 8, :vl],
                    stage[0:128:32, :, :vl],
                )

            # epilogue: exp (+denominator via accum) on the group's 16 slot
            # rows, 16-wide transposes, AV with per-group PSUM accumulation
            # into av[0:16], scaled rows DMA'd out to their slot offset
            l_g = work.tile([128, 1], F32, tag="l", name="l_g")
            exp_g = work.tile([128, kw128], BF16, tag="exp", name="exp_g")
            nc.scalar.activation(
                exp_g[0:QPC, :],
                scores_g[0:QPC, :],
                mybir.ActivationFunctionType.Exp,
                accum_out=l_g[0:QPC],
            )
            rl = work.tile([128, 1], F32, tag="rl", name="rl")
            nc.vector.reciprocal(rl[0:QPC], l_g[0:QPC])
            ntile = kw128 // 128
            for t in range(ntile):
                kt = off // 128 + t
                vt = work.tile([128, V], BF16, tag="vt", bufs=4, name="vt")
                nc.sync.dma_start(vt[:], vext[kt * 128 : (kt + 1) * 128, :])
                pt = psum.tile([128, 128], BF16, tag="misc", bufs=2, name="pt")
                nc.tensor.transpose(
                    pt[:, 0:QPC],
                    exp_g[0:QPC, t * 128 : (t + 1) * 128],
                    ident[0:QPC, 0:QPC],
                )
                expt = work.tile([128, QPC], BF16, tag="expT", bufs=4, name="expt")
                nc.vector.tensor_copy(expt[:], pt[:, 0:QPC])
                nc.tensor.matmul(
                    av[0:QPC, 0:V],
                    lhsT=expt[:],
                    rhs=vt[:],
                    start=(t == 0),
                    stop=(t == ntile - 1),
                )
            osb = work.tile([128, V], F32, tag="osb", bufs=2, name="osb")
            nc.vector.tensor_scalar_mul(
                osb[0:QPC, :], av[0:QPC, 0:V], rl[0:QPC]
            )
            nc.sync.dma_start(out.ap()[r0g : r0g + QPC, :], osb[0:QPC, :])

    nc.compile()
    return nc


_CACHE = {}


def _install_profile_shim():
    """Provide antenv.axon_hooks (absent in this image) so
    run_bass_kernel_spmd(trace=True) can capture NTFF profiles through
    libaxon_pjrt.so, mirroring trn_agent_boot's bootstrap."""
    import types

    if "antenv.axon_hooks" not in sys.modules:
        mod = types.ModuleType("antenv.axon_hooks")
        state = {}
        mod.set_axon_ntff_profile_hook = lambda h: state.__setitem__("h", h)
        mod.get_axon_ntff_profile_hook = lambda: state.get("h")
        sys.modules["antenv.axon_hooks"] = mod
        import antenv

        antenv.axon_hooks = mod
        if "/root/.axon_site" not in sys.path:
            sys.path.insert(0, "/root/.axon_site")
        from trn_agent_boot.trn_boot import _ntff_profile_via_ctypes

        hook = _ntff_profile_via_ctypes("/opt/axon/libaxon_pjrt.so")
        mod.set_axon_ntff_profile_hook(hook)

        import concourse.bass_utils as bu

        orig_upload = bu.upload_artifacts

        def _safe_upload(tmpdir):
            try:
                return orig_upload(tmpdir)
            except Exception:
                return f"local:{tmpdir}"

        bu.upload_artifacts = _safe_upload


def _get_graph(valid_lens):
    key = tuple(int(v) for v in valid_lens)
    if _CACHE.get("key") != key:
        _CACHE["nc"] = _build_graph(valid_lens)
        _CACHE["key"] = key
    return _CACHE["nc"]


def _make_in_maps(queries, keys, values, valid_lens):
    order, vls, _vl8s, kws, koff, KEXT = _kw_template(valid_lens)
    kT = np.zeros((D, KEXT), dtype=np.float32)
    vext = np.zeros((KEXT, V), dtype=np.float32)
    for g, b in enumerate(order):
        vl, kw, off = vls[g], kws[g], int(koff[g])
        kT[:, off : off + kw] = keys[b, :kw].T
        vext[off : off + vl, :] = values[b, :vl]
    kT_bf = kT.astype(NP_BF16)
    vext_bf = vext.astype(NP_BF16)
    in_maps = []
    for c in range(B):
        qrows = np.concatenate(
            [queries[b, c * QPC : (c + 1) * QPC] for b in order], axis=0
        )  # [128, D]; slot 16*g + r = (batch order[g], row 16*c + r)
        w = np.concatenate(
            [qrows.T, _CACHE["wq_f"], _CACHE["wk_f"]], axis=1
        )  # [D, Q+2H]
        in_maps.append(
            {
                "w": w.astype(NP_BF16),
                "kT": kT_bf,
                "vext": vext_bf,
                "wv2": _CACHE["wv2_bf"],
            }
        )
    return in_maps


def kernel(
    queries, keys, values, valid_lens, Wq, Wk, wv, _profile=False, **_unused
):
    queries = np.asarray(queries, dtype=np.float32)
    keys = np.asarray(keys, dtype=np.float32)
    values = np.asarray(values, dtype=np.float32)
    valid_lens = np.asarray(valid_lens)
    _CACHE["wq_f"] = np.asarray(Wq, np.float32)
    _CACHE["wk_f"] = np.asarray(Wk, np.float32)
    _CACHE["wv2_bf"] = (
        np.asarray(wv, np.float32).reshape(HT, 128).T.copy().astype(NP_BF16)
    )

    nc = _get_graph(valid_lens)
    in_maps = _make_in_maps(queries, keys, values, valid_lens)
    kwargs = {}
    if _profile:
        _install_profile_shim()
        tdir = "/root/problem/trace_out"
        os.makedirs(tdir, exist_ok=True)
        kwargs["tmpdir"] = tdir
    res = run_bass_kernel_spmd(
        nc, in_maps, core_ids=list(range(B)), trace=_profile, **kwargs
    )
    order = _kw_template(valid_lens)[0]
    out = np.zeros((B, Q, V), dtype=np.float32)
    for c in range(B):
        oc = np.asarray(res.results[c]["out"], dtype=np.float32)
        for g, b in enumerate(order):
            out[b, c * QPC : (c + 1) * QPC] = oc[g * QPC : (g + 1) * QPC]
    if _profile:
        _CACHE["last_result"] = res
    return out


# revision 21
# speedup vs baseline: 4.4016x; 1.0320x over previous
"""Additive attention (Bahdanau) Trainium2 kernel, SPMD over 8 NeuronCores.

Math per batch b (see reference):
    q = queries[b] @ Wq                  [Q=128, H=256]
    k = keys[b]    @ Wk                  [K=1024, H=256]
    scores[i,j] = sum_h wv[h] * tanh(q[i,h] + k[j,h])
    attn = masked_softmax(scores, valid_len[b])
    out[b] = attn @ values[b]            [Q, V=512]

Sharding: sequence-parallel q-striping. Each core takes 16 q-rows of EVERY
batch and only the valid k-range of each batch. Per-core work is perfectly
balanced for any valid_lens; no collectives (softmax is per-q-row).

The kernel is ACT-bound: tanh over ~16*sum(vl)*256 elements per core runs at
1 elem/cycle/lane (153.6 G/s); everything else is scheduled around keeping
the scalar engine 100% busy:
  - group order: smallest batch first (fast ramp while kT streams in),
    middle ascending, 2nd-smallest last (short epilogue tail)
  - one packed [qT|Wq|Wk] input DMA; group-aligned kT segments
  - S-build adds (DVE tensor_scalar, 2x bf16) with a slice of rows on
    GPSIMD to keep DVE under the ACT time; h-tiles merged in one tile so
    each (g,jg) chunk is a single big tanh instruction
  - scores via M=1 matmuls at PE column tiles {0,32,64,96}, two row-halves
    per column tile -> one [128,2,kw] PSUM slab per chunk, one DVE drain
  - exp with accum_out produces the softmax denominator for free (masking
    comes from exp/accum covering exactly [:vl] and host-zeroed values)
  - per-group AV accumulation (start/stop per group) + per-group out DMA:
    no NEG filler needed, tail is only the last (2nd-smallest) group
"""

import os
import sys

import numpy as np

for _p in ("/opt/trn_rl_repo", "/root/.axon_site/_ro/trn_rl_repo"):
    if os.path.isdir(_p) and _p not in sys.path:
        sys.path.insert(0, _p)

os.environ.setdefault("MYCRO_LOCAL_CACHE", "1")

import ml_dtypes  # noqa: E402
from contextlib import ExitStack  # noqa: E402

import concourse.bass as bass  # noqa: E402
import concourse.tile as tile  # noqa: E402
from concourse import bacc, mybir  # noqa: E402
from concourse.bass_utils import run_bass_kernel_spmd  # noqa: E402
from concourse.masks import make_identity  # noqa: E402

BF16 = mybir.dt.bfloat16
F32 = mybir.dt.float32
NP_BF16 = ml_dtypes.bfloat16

B, Q, K, D, H, V = 8, 128, 1024, 512, 256, 512
DC = D // 128   # 4 contraction tiles for the projections
HT = H // 128   # 2 h-tiles
QPC = Q // B    # 16 q-rows per (batch, core)
ACT_ROWS = 0    # rows per 8-row chunk done as tanh(kh + bias) on ACT
                # (costs +2x224cyc on ACT, saves 2x(58+vl/2)cyc on DVE --
                # balances the two near-critical engines)


def _kw_template(valid_lens):
    """Group order: smallest batch first (fast pipeline start while kT is
    still streaming), the rest ascending, 2nd-smallest last (short epilogue
    tail). Returns per-GROUP (batch, vl, vl8, kw128, koff)."""
    vls = [max(1, min(K, int(v))) for v in valid_lens]
    by = sorted(range(len(vls)), key=lambda b: (vls[b], b))
    order = [by[0]] + by[2:] + [by[1]]
    vl = [vls[b] for b in order]
    vl8 = [-(-v // 8) * 8 for v in vl]
    kw = [-(-v // 128) * 128 for v in vl]
    koff = np.concatenate([[0], np.cumsum(kw)]).astype(int)
    return order, vl, vl8, kw, koff, int(koff[-1])


def _build_graph(valid_lens):
    _order, vls, vl8s, kws, koff, KEXT = _kw_template(valid_lens)
    nc = bacc.Bacc(
        "TRN2",
        target_bir_lowering=False,
        debug=False,
        num_devices=8,
    )

    # w = [qT | Wq | Wk] packed along columns (all have leading dim D)
    WCOLS = Q + 2 * H
    w = nc.dram_tensor("w", [D, WCOLS], BF16, kind="ExternalInput")
    kT = nc.dram_tensor("kT", [D, KEXT], BF16, kind="ExternalInput")
    vext = nc.dram_tensor("vext", [KEXT, V], BF16, kind="ExternalInput")
    wv2 = nc.dram_tensor("wv2", [128, HT], BF16, kind="ExternalInput")
    out = nc.dram_tensor("out", [Q, V], F32, kind="ExternalOutput")

    # kT segments, group-aligned so each group's projection lands as soon
    # as its own columns arrive (no waiting on later groups' data)
    gsegs = []
    for g in range(B):
        off = int(koff[g])
        gsegs.append([])
        while off < int(koff[g + 1]):
            cw = min(512, int(koff[g + 1]) - off)
            gsegs[g].append((off, cw))
            off += cw

    with tile.TileContext(nc) as tc, ExitStack() as ctx:
        singles = ctx.enter_context(tc.tile_pool(name="singles", bufs=1))
        work = ctx.enter_context(tc.tile_pool(name="work", bufs=2))
        psum = ctx.enter_context(tc.tile_pool(name="psum", bufs=1, space="PSUM"))

        # tiny warmup activation so the ~2.7us ACT table load overlaps DMAs
        warm = singles.tile([1, 2], F32)
        nc.vector.memset(warm[:], 0.0)
        nc.scalar.activation(warm[:], warm[:], mybir.ActivationFunctionType.Tanh)

        # ---- input DMAs ------------------------------------------------
        w_sb = singles.tile([128, DC, WCOLS], BF16)
        nc.sync.dma_start(w_sb[:], w.ap().rearrange("(c p) n -> p c n", p=128))
        wv_sb = singles.tile([128, HT], BF16)
        nc.sync.dma_start(wv_sb[:], wv2.ap())
        ident = singles.tile([128, 128], BF16)
        make_identity(nc, ident[:])
        # warm the PE HAM clock gate during the input DMAs so the projection
        # matmuls run at full clock
        pwarm = psum.tile([128, 512], F32, tag="misc", bufs=2, name="pwarm")
        for i in range(8):
            nc.tensor.matmul(
                pwarm[:, :128],
                lhsT=ident[:],
                rhs=ident[:],
                start=(i == 0),
                stop=(i == 7),
            )

        # ---- projections: qh [128, HT, Q] f32, kh [128, HT, KEXT] bf16 --
        qh_sb = singles.tile([128, HT, Q], F32)
        kh_sb = singles.tile([128, HT, KEXT], BF16)

        def _proj(dst, rhs, n, wcol0, ht):
            ps = psum.tile([128, 512], F32, tag="misc", bufs=2, name="ps")
            for dc in range(DC):
                nc.tensor.matmul(
                    ps[:, :n],
                    lhsT=w_sb[:, dc, wcol0 + ht * 128 : wcol0 + (ht + 1) * 128],
                    rhs=rhs(dc),
                    start=(dc == 0),
                    stop=(dc == DC - 1),
                )
            nc.vector.tensor_copy(dst, ps[:, :n])

        for ht in range(HT):
            _proj(qh_sb[:, ht, :], lambda dc: w_sb[:, dc, 0:Q], Q, Q, ht)

        kt_r = kT.ap().rearrange("(c p) k -> p c k", p=128)

        def _emit_group_segs(gi):
            for soff, scw in gsegs[gi]:
                ktc = work.tile(
                    [128, DC, 512], BF16, tag="ktc", bufs=2, name="ktc"
                )
                nc.sync.dma_start(ktc[:, :, :scw], kt_r[:, :, soff : soff + scw])
                for ht in range(HT):
                    _proj(
                        kh_sb[:, ht, soff : soff + scw],
                        lambda dc: ktc[:, dc, :scw],
                        scw,
                        Q + H,
                        ht,
                    )

        # ---- per-group compute ------------------------------------------
        slab = psum.tile([128, 2, 1024], F32, tag="slab", name="slab")
        nc.vector.memset(slab[:], 0.0)
        av = psum.tile([128, V], F32, tag="av", name="av")

        # segments are emitted interleaved with the group loop (need-order:
        # the scheduler keeps each engine's queue roughly in emission order,
        # so later groups' kh casts must not sit ahead of earlier adds);
        # each group's epilogue is emitted after the NEXT group's first
        # chunk so the PE queue never stalls on a not-yet-drained exp
        _emit_group_segs(0)
        seg_emitted = 1
        epi_pending = None

        def _epilogue(g):
            vl, vl8, off = vls[g], vl8s[g], int(koff[g])
            kw128 = kws[g]
            r0g = g * QPC
            scores_g = score_tiles[g]
            l_g = work.tile([128, 1], F32, tag="l", name="l_g")
            exp_g = work.tile([128, kw128], BF16, tag="exp", name="exp_g")
            nc.scalar.activation(
                exp_g[0:QPC, :],
                scores_g[0:QPC, :],
                mybir.ActivationFunctionType.Exp,
                accum_out=l_g[0:QPC],
            )
            rl = work.tile([128, 1], F32, tag="rl", name="rl")
            nc.vector.reciprocal(rl[0:QPC], l_g[0:QPC])
            ntile = kw128 // 128
            for t in range(ntile):
                kt = off // 128 + t
                vt = work.tile([128, V], BF16, tag="vt", bufs=4, name="vt")
                nc.sync.dma_start(vt[:], vext[kt * 128 : (kt + 1) * 128, :])
                pt = psum.tile([128, 128], BF16, tag="misc", bufs=2, name="pt")
                nc.tensor.transpose(
                    pt[:, 0:QPC],
                    exp_g[0:QPC, t * 128 : (t + 1) * 128],
                    ident[0:QPC, 0:QPC],
                )
                expt = work.tile([128, QPC], BF16, tag="expT", bufs=4, name="expt")
                nc.vector.tensor_copy(expt[:], pt[:, 0:QPC])
                nc.tensor.matmul(
                    av[0:QPC, 0:V],
                    lhsT=expt[:],
                    rhs=vt[:],
                    start=(t == 0),
                    stop=(t == ntile - 1),
                )
            osb = work.tile([128, V], F32, tag="osb", bufs=2, name="osb")
            nc.vector.tensor_scalar_mul(
                osb[0:QPC, :], av[0:QPC, 0:V], rl[0:QPC]
            )
            nc.sync.dma_start(out.ap()[r0g : r0g + QPC, :], osb[0:QPC, :])

        score_tiles = {}
        for g in range(B):
            vl, vl8, off = vls[g], vl8s[g], int(koff[g])
            kw128 = kws[g]
            r0g = g * QPC
            # the group's 16 slot rows land at partitions 0:16 (the drain
            # DMA regathers); the whole epilogue runs at base partition 0
            # and the out DMA scatters rows back to their slot offset
            scores_g = work.tile([128, kw128], F32, tag="scores", name="scores_g")
            score_tiles[g] = scores_g
            if vl < kw128:
                # pad columns feed the denominator accum -> push them to ~0
                nc.vector.memset(scores_g[0:QPC, vl:kw128], -60.0)
            for jg in range(QPC // 8):
                r0 = g * QPC + jg * 8
                st = work.tile(
                    [128, HT, 8, vl8], BF16, tag="st", bufs=3, name="st"
                )
                nd = 8 - ACT_ROWS  # rows 0..nd added on DVE, rest via bias
                for j in range(nd):
                    for ht in range(HT):
                        nc.vector.tensor_scalar_add(
                            st[:, ht, j, :],
                            kh_sb[:, ht, off : off + vl8],
                            qh_sb[:, ht, r0 + j : r0 + j + 1],
                        )
                # bias rows first: their input (kh) is ready before the DVE
                # adds finish, so ACT can run them while DVE works
                for j in range(nd, 8):
                    for ht in range(HT):
                        nc.scalar.activation(
                            st[:, ht, j, :],
                            kh_sb[:, ht, off : off + vl8],
                            mybir.ActivationFunctionType.Tanh,
                            bias=qh_sb[:, ht, r0 + j : r0 + j + 1],
                        )
                if g == 0 and jg == 0:
                    # split so ACT starts after only 4 rows' worth of adds
                    nc.scalar.activation(
                        st[:, :, 0:4, :], st[:, :, 0:4, :],
                        mybir.ActivationFunctionType.Tanh,
                    )
                    nc.scalar.activation(
                        st[:, :, 4:nd, :], st[:, :, 4:nd, :],
                        mybir.ActivationFunctionType.Tanh,
                    )
                else:
                    nc.scalar.activation(
                        st[:, :, 0:nd, :], st[:, :, 0:nd, :],
                        mybir.ActivationFunctionType.Tanh,
                    )
                # scores: M=1 matmuls; row j -> PE column tile 32*(j//2),
                # slab half j%2 (so the strided drain lands rows in order)
                for ht in range(HT):
                    for j in range(8):
                        j4, half = j >> 1, j & 1
                        for c in range(0, vl8, 512):
                            cw = min(512, vl8 - c)
                            nc.tensor.matmul(
                                slab[32 * j4 : 32 * j4 + 1, half, c : c + cw],
                                lhsT=wv_sb[:, ht : ht + 1],
                                rhs=st[:, ht, j, c : c + cw],
                                start=(ht == 0),
                                stop=(ht == 1),
                                tile_position=(0, 32 * j4),
                            )
                stage = work.tile(
                    [128, 2, vl8], F32, tag="stage", bufs=2, name="stage"
                )
                nc.vector.tensor_copy(stage[:, :, :vl], slab[:, :, :vl])
                nc.sync.dma_start(
                    scores_g[r0 - r0g : r0 - r0g + 8, :vl],
                    stage[0:128:32, :, :vl],
                )
                if jg == 0:
                    if epi_pending is not None:
                        _epilogue(epi_pending)
                        epi_pending = None
                    while seg_emitted <= g + 1 and seg_emitted < B:
                        _emit_group_segs(seg_emitted)
                        seg_emitted += 1
            epi_pending = g
            while seg_emitted <= g + 2 and seg_emitted < B:
                _emit_group_segs(seg_emitted)
                seg_emitted += 1
        _epilogue(epi_pending)

    nc.compile()
    return nc


_CACHE = {}


def _install_profile_shim():
    """Provide antenv.axon_hooks (absent in this image) so
    run_bass_kernel_spmd(trace=True) can capture NTFF profiles through
    libaxon_pjrt.so, mirroring trn_agent_boot's bootstrap."""
    import types

    if "antenv.axon_hooks" not in sys.modules:
        mod = types.ModuleType("antenv.axon_hooks")
        state = {}
        mod.set_axon_ntff_profile_hook = lambda h: state.__setitem__("h", h)
        mod.get_axon_ntff_profile_hook = lambda: state.get("h")
        sys.modules["antenv.axon_hooks"] = mod
        import antenv

        antenv.axon_hooks = mod
        if "/root/.axon_site" not in sys.path:
            sys.path.insert(0, "/root/.axon_site")
        from trn_agent_boot.trn_boot import _ntff_profile_via_ctypes

        hook = _ntff_profile_via_ctypes("/opt/axon/libaxon_pjrt.so")
        mod.set_axon_ntff_profile_hook(hook)

        import concourse.bass_utils as bu

        orig_upload = bu.upload_artifacts

        def _safe_upload(tmpdir):
            try:
                return orig_upload(tmpdir)
            except Exception:
                return f"local:{tmpdir}"

        bu.upload_artifacts = _safe_upload


def _get_graph(valid_lens):
    key = tuple(int(v) for v in valid_lens)
    if _CACHE.get("key") != key:
        _CACHE["nc"] = _build_graph(valid_lens)
        _CACHE["key"] = key
    return _CACHE["nc"]


def _make_in_maps(queries, keys, values, valid_lens):
    order, vls, _vl8s, kws, koff, KEXT = _kw_template(valid_lens)
    kT = np.zeros((D, KEXT), dtype=np.float32)
    vext = np.zeros((KEXT, V), dtype=np.float32)
    for g, b in enumerate(order):
        vl, kw, off = vls[g], kws[g], int(koff[g])
        kT[:, off : off + kw] = keys[b, :kw].T
        vext[off : off + vl, :] = values[b, :vl]
    kT_bf = kT.astype(NP_BF16)
    vext_bf = vext.astype(NP_BF16)
    in_maps = []
    for c in range(B):
        qrows = np.concatenate(
            [queries[b, c * QPC : (c + 1) * QPC] for b in order], axis=0
        )  # [128, D]; slot 16*g + r = (batch order[g], row 16*c + r)
        w = np.concatenate(
            [qrows.T, _CACHE["wq_f"], _CACHE["wk_f"]], axis=1
        )  # [D, Q+2H]
        in_maps.append(
            {
                "w": w.astype(NP_BF16),
                "kT": kT_bf,
                "vext": vext_bf,
                "wv2": _CACHE["wv2_bf"],
            }
        )
    return in_maps


def kernel(
    queries, keys, values, valid_lens, Wq, Wk, wv, _profile=False, **_unused
):
    queries = np.asarray(queries, dtype=np.float32)
    keys = np.asarray(keys, dtype=np.float32)
    values = np.asarray(values, dtype=np.float32)
    valid_lens = np.asarray(valid_lens)
    _CACHE["wq_f"] = np.asarray(Wq, np.float32)
    _CACHE["wk_f"] = np.asarray(Wk, np.float32)
    _CACHE["wv2_bf"] = (
        np.asarray(wv, np.float32).reshape(HT, 128).T.copy().astype(NP_BF16)
    )

    nc = _get_graph(valid_lens)
    in_maps = _make_in_maps(queries, keys, values, valid_lens)
    kwargs = {}
    if _profile:
        _install_profile_shim()
        tdir = "/root/problem/trace_out"
        os.makedirs(tdir, exist_ok=True)
        kwargs["tmpdir"] = tdir
    res = run_bass_kernel_spmd(
        nc, in_maps, core_ids=list(range(B)), trace=_profile, **kwargs
    )
    order = _kw_template(valid_lens)[0]
    out = np.zeros((B, Q, V), dtype=np.float32)
    for c in range(B):
        oc = np.asarray(res.results[c]["out"], dtype=np.float32)
        for g, b in enumerate(order):
            out[b, c * QPC : (c + 1) * QPC] = oc[g * QPC : (g + 1) * QPC]
    if _profile:
        _CACHE["last_result"] = res
    return out
